# revision 16
# baseline (speedup 1.0000x reference)
"""MoE fused top-k-gating decode kernel for Trainium2 (8 NeuronCores).

Problem: B=32,S=1,H=2048, E=8 experts, I=5632, top_k=2, fp32.
Sharding: expert-parallel - core c owns expert c (w_gate/w_up/w_down[c]),
router weight replicated (rolled per-core so column 0 is the own expert).
Each core computes the full router (softmax + top-2 mask), its expert's
GLU-MLP for all 32 tokens, scales by its combine weight, and returns a
partial [T,H] output (fp16); the host sums the 8 partials.

The kernel is DMA-bound: weights stream as fp16 (host-side cast;
end-to-end rel err ~5e-4 vs the 2e-2 gate): 69.2 MB @ ~360 GB/s =
~193 us/core floor. Host prepacks every tensor into its SBUF tile
layout so each weight DMA is 128 contiguous >=4KB descriptors.

Tail scheduling: down-weight chunks are streamed shifted - slab n's
epilogue feeds chunks 4n..4n+3, but chunks 0 and 1 (whose interT is
ready after slab 0) are streamed LAST, so the dependent chain after
the final weight byte is just 4 matmuls + output copy instead of a
full slab epilogue. The combine weight is folded into the
intermediate before the down matmul, so the output needs no final
scale; copies out of PSUM alternate DVE/Pool to overlap.
"""

import numpy as np

import concourse.bass as bass
import concourse.bacc as bacc
import concourse.mybir as mybir
import concourse.tile as tile
from concourse.masks import make_identity
from concourse.bass_utils import run_bass_kernel_spmd

B, S, H = 32, 1, 2048
E, I = 8, 5632
T = B * S          # 32 tokens
P = 128            # partitions
NCORES = 8
SWIGLU_SCALE = 1.702

KH = H // P        # 16 contraction chunks over H
KI = I // P        # 44 contraction chunks over I
NW = 512           # moving-dim tile width
NT = I // NW       # 11 gate/up column slabs
ND = H // NW       # 4 down output tiles
XW = T + E         # packed xT+router width (40)
SLAB = KH * NW     # 8192 cols per gate/up slab tile

F32 = mybir.dt.float32
F16 = mybir.dt.float16
AX = mybir.AxisListType.X
AF = mybir.ActivationFunctionType
OP = mybir.AluOpType


def _build_nc() -> bass.Bass:
    nc = bacc.Bacc()

    WDT = F16
    xrw_d = nc.declare_dram_parameter("xrw", [P, KH * XW], WDT, isOutput=False)
    wg_d = nc.declare_dram_parameter("wg", [H, I], WDT, isOutput=False)
    wu_d = nc.declare_dram_parameter("wu", [H, I], WDT, isOutput=False)
    wd_d = nc.declare_dram_parameter("wd", [I, H], WDT, isOutput=False)
    out_d = nc.declare_dram_parameter("out", [T, H], F16, isOutput=True)

    with tile.TileContext(nc) as tc:
        with tc.tile_pool(name="const", bufs=1) as const:
            id_sb = const.tile([T, T], F32, name="id_sb")
            make_identity(nc, id_sb)

            xrw_sb = const.tile([P, KH * XW], WDT, name="xrw_sb")
            nc.sync.dma_start(out=xrw_sb, in_=xrw_d[:, :])

            interT_sb = const.tile([P, KI * T], WDT, name="interT_sb")
            comb_sb = const.tile([T, 1], F32, name="comb_sb")
            out_sb = const.tile([T, H], F16, name="out_sb")

            def xT_k(k):  # [128, 32] stationary activation chunk
                return xrw_sb[:, k * XW : k * XW + T]

            def rw_k(k):  # [128, 8] router weight chunk
                return xrw_sb[:, k * XW + T : (k + 1) * XW]

            # weight DMAs keep ~1-4KB descriptors: real HW moves small
            # descriptors at full rate, big (16KB) ones ~12% slower
            wg_cols = wg_d.rearrange("(k p) i -> p k i", p=P)
            wu_cols = wu_d.rearrange("(k p) i -> p k i", p=P)
            wd_rows = wd_d.rearrange("(q p) h -> p q h", p=P)
            wgp = tc.alloc_tile_pool(name="wgp", bufs=3)
            wup = tc.alloc_tile_pool(name="wup", bufs=3)
            wdp = tc.alloc_tile_pool(name="wdp", bufs=3)

            # ---------------- router: softmax + top-2 mask ----------------
            with (
                tc.tile_pool(name="rps", bufs=1, space="PSUM") as rps,
                tc.tile_pool(name="rsb", bufs=1) as rsb,
            ):
                # absorb the ident DMA tick on PE before anything else
                dmy_ps = rps.tile([T, T], F32, name="dmy_ps", tag="dmy")
                nc.tensor.transpose(dmy_ps, id_sb, id_sb)

                logits = rps.tile([T, E], F32, name="logits", tag="logits")
                for k in range(KH):
                    nc.tensor.matmul(
                        logits,
                        xT_k(k),
                        rw_k(k),
                        start=(k == 0),
                        stop=(k == KH - 1),
                    )
                # PSUM is read only by DVE (keeps later PE writers 1-wait)
                lg = rsb.tile([T, E], F32, name="lg")
                nc.vector.tensor_copy(lg, logits)
                mx = rsb.tile([T, 1], F32, name="mx")
                nc.vector.reduce_max(mx, lg, axis=AX)
                nmx = rsb.tile([T, 1], F32, name="nmx")
                nc.vector.tensor_scalar_mul(nmx, mx, -1.0)
                ex = rsb.tile([T, E], F32, name="ex")
                nc.scalar.activation(ex, lg, AF.Exp, bias=nmx, scale=1.0)
                sm = rsb.tile([T, 1], F32, name="sm")
                nc.vector.reduce_sum(sm, ex, axis=AX)
                rc = rsb.tile([T, 1], F32, name="rc")
                nc.vector.reciprocal(rc, sm)
                aff = rsb.tile([T, E], F32, name="aff")
                nc.vector.tensor_scalar_mul(aff, ex, rc)
                # top-2: value >= (second largest)
                m1 = rsb.tile([T, 1], F32, name="m1")
                nc.vector.reduce_max(m1, aff, axis=AX)
                pen = rsb.tile([T, E], F32, name="pen")
                nc.vector.tensor_scalar(
                    pen, aff, m1, -1e30, op0=OP.is_equal, op1=OP.mult
                )
                b2 = rsb.tile([T, E], F32, name="b2")
                nc.vector.tensor_add(b2, aff, pen)
                m2 = rsb.tile([T, 1], F32, name="m2")
                nc.vector.reduce_max(m2, b2, axis=AX)
                ge = rsb.tile([T, E], F32, name="ge")
                nc.vector.tensor_scalar(ge, aff, m2, None, op0=OP.is_ge)
                msk = rsb.tile([T, E], F32, name="msk")
                nc.vector.tensor_mul(msk, aff, ge)
                # rolled router weight puts the own expert at column 0.
                # ACT-engine Copy also prewarms its table for the tail copies.
                nc.scalar.activation(comb_sb, msk[:, 0:1], AF.Copy, scale=1.0)

            # ---- fused gate/up + swiglu + transpose + interleaved down ----
            # Slab n computes interT chunks 4n..4n+3. Down matmuls run on
            # the freshest interT: slab 0 feeds chunks 2,3 (pair on gpsimd);
            # slabs 1..10 feed their own 4 chunks (2 pairs each, gpsimd);
            # chunks 0 (sync) and 1 (gpsimd) stream LAST so the post-stream
            # dependent chain is only their 8 matmuls + output copies.
            # Accumulation order per PSUM bank: 2,3,4..43,0,1 (start at 2,
            # stop at 1). PSUM: gate/up 2 + transpose 2 + down accum 4 = 8.
            with (
                tc.tile_pool(name="gup", bufs=1, space="PSUM") as gup,
                tc.tile_pool(name="tps", bufs=2, space="PSUM") as tps,
                tc.tile_pool(name="dps", bufs=1, space="PSUM") as dps,
                tc.tile_pool(name="esb", bufs=2) as esb,
            ):
                d_ps = [
                    dps.tile([T, NW], F32, name=f"d_ps{j}", tag=f"d{j}")
                    for j in range(ND)
                ]

                def down_mms(ki, wd_ap, c):
                    # 4 matmuls accumulating chunk ki from wd_ap's chunk c
                    for j in range(ND):
                        nc.tensor.matmul(
                            d_ps[j],
                            interT_sb[:, ki * T : (ki + 1) * T],
                            wd_ap[:, c * H + j * NW : c * H + (j + 1) * NW],
                            start=(ki == 2),
                            stop=(ki == 1),
                        )

                widths = [NW] * (NT - 1) + [NW // 2, NW // 2]
                assert sum(widths) == I
                c0 = 0
                for n, w in enumerate(widths):
                    wg_sl = wgp.tile([P, SLAB], WDT, name="wg_sl", tag="wg")
                    wu_sl = wup.tile([P, SLAB], WDT, name="wu_sl", tag="wu")
                    # wg on the sync queue, wu on the scalar queue: two
                    # independent issue pipelines keep the shared DMA
                    # engines fed across sem-prop/DGE latency holes.
                    if w == NW:
                        khs = [slice(0, KH)]
                    else:
                        # tail slabs stream in k-halves so matmuls overlap
                        khs = [slice(0, KH // 2), slice(KH // 2, KH)]
                    for kh in khs:
                        nc.sync.dma_start(
                            out=wg_sl.rearrange("p (k c) -> p k c", c=NW)[
                                :, kh, :w
                            ],
                            in_=wg_cols[:, kh, c0 : c0 + w],
                        )
                    for kh in khs:
                        nc.scalar.dma_start(
                            out=wu_sl.rearrange("p (k c) -> p k c", c=NW)[
                                :, kh, :w
                            ],
                            in_=wu_cols[:, kh, c0 : c0 + w],
                        )
                    g_ps = gup.tile([T, NW], F32, name="g_ps", tag="g")
                    u_ps = gup.tile([T, NW], F32, name="u_ps", tag="u")
                    for k in range(KH):
                        nc.tensor.matmul(
                            g_ps[:, :w],
                            xT_k(k),
                            wg_sl[:, k * NW : k * NW + w],
                            start=(k == 0),
                            stop=(k == KH - 1),
                        )
                    for k in range(KH):
                        nc.tensor.matmul(
                            u_ps[:, :w],
                            xT_k(k),
                            wu_sl[:, k * NW : k * NW + w],
                            start=(k == 0),
                            stop=(k == KH - 1),
                        )
                    # epilogue: sigmoid runs off a copy; fold combine weight
                    g_sb = esb.tile([T, NW], F32, name="g_sb", tag="gsb")
                    nc.vector.tensor_copy(g_sb[:, :w], g_ps[:, :w])
                    sig = esb.tile([T, NW], F32, name="sig", tag="sig")
                    nc.scalar.activation(
                        sig[:, :w], g_sb[:, :w], AF.Sigmoid, scale=SWIGLU_SCALE
                    )
                    t1 = esb.tile([T, NW], F32, name="t1", tag="t1")
                    nc.vector.tensor_mul(t1[:, :w], g_ps[:, :w], sig[:, :w])
                    t2 = esb.tile([T, NW], F32, name="t2", tag="t2")
                    nc.vector.tensor_mul(t2[:, :w], t1[:, :w], u_ps[:, :w])
                    inter = esb.tile([T, NW], F32, name="inter", tag="inter")
                    nc.vector.tensor_scalar_mul(
                        inter[:, :w], t2[:, :w], comb_sb
                    )
                    for j in range(w // P):
                        ic = c0 // P + j
                        tp = tps.tile([P, T], F32, name="tp", tag="tp")
                        nc.tensor.transpose(tp, inter[:, j * P : (j + 1) * P], id_sb)
                        nc.vector.tensor_copy(
                            interT_sb[:, ic * T : (ic + 1) * T], tp
                        )
                    # down-weight pairs + matmuls for this slab's chunks
                    # (chunks 0,1 deferred to the end of the stream)
                    kis = [ki for ki in range(c0 // P, (c0 + w) // P)
                           if ki not in (0, 1)]
                    for i in range(0, len(kis), 2):
                        k0, k1 = kis[i], kis[i + 1]
                        wd_pr = wdp.tile([P, 2 * H], WDT, name="wd_pr", tag="wdpr")
                        nc.gpsimd.dma_start(
                            out=wd_pr.rearrange("p (q h) -> p q h", h=H),
                            in_=wd_rows[:, k0 : k1 + 1, :],
                        )
                        down_mms(k0, wd_pr, 0)
                        down_mms(k1, wd_pr, 1)
                    c0 += w

                # final chunks 0 (sync queue) and 1 (gpsimd queue)
                wd_c0 = wdp.tile([P, H], WDT, name="wd_c0", tag="wds0")
                nc.sync.dma_start(out=wd_c0, in_=wd_d[0:P, :])
                wd_c1 = wdp.tile([P, H], WDT, name="wd_c1", tag="wds1")
                nc.gpsimd.dma_start(out=wd_c1, in_=wd_d[P : 2 * P, :])
                down_mms(0, wd_c0, 0)
                down_mms(1, wd_c1, 0)

                # output: PSUM -> fp16 SBUF (alternate DVE/ACT) -> DRAM
                for j in range(ND):
                    if j % 2 == 0:
                        nc.vector.tensor_copy(
                            out_sb[:, j * NW : (j + 1) * NW], d_ps[j]
                        )
                    else:
                        nc.scalar.activation(
                            out_sb[:, j * NW : (j + 1) * NW], d_ps[j],
                            AF.Copy, scale=1.0,
                        )
                    nc.sync.dma_start(
                        out=out_d[:, j * NW : (j + 1) * NW],
                        in_=out_sb[:, j * NW : (j + 1) * NW],
                    )
            wdp.release()
            wup.release()
            wgp.release()
    nc.finalize()
    return nc


def _pack_rows(a: np.ndarray) -> np.ndarray:
    """[K*P, C] row-major -> [P, K*C] SBUF tile layout (fp16)."""
    kp, c = a.shape
    k = kp // P
    return np.ascontiguousarray(
        a.reshape(k, P, c).transpose(1, 0, 2).reshape(P, k * c)
    )


def _make_in_maps(hidden_states, router_weight, w_gate, w_up, w_down):
    x = np.asarray(hidden_states, np.float32).reshape(T, H)
    rw = np.asarray(router_weight, np.float32)
    wg = np.asarray(w_gate, np.float16)
    wu = np.asarray(w_up, np.float16)
    wd = np.asarray(w_down, np.float16)
    xT = x.T.astype(np.float16)  # [H, T]

    in_maps = []
    for c in range(NCORES):
        order = [(j + c) % E for j in range(E)]  # column j holds expert (j+c)%E
        rwT = rw[order].T.astype(np.float16)  # [H, E]; col 0 = own expert
        xrw = _pack_rows(
            np.ascontiguousarray(np.concatenate([xT, rwT], axis=1))
        )  # [P, KH*XW]
        in_maps.append(
            {
                "xrw": xrw,
                "wg": np.ascontiguousarray(wg[c]),
                "wu": np.ascontiguousarray(wu[c]),
                "wd": np.ascontiguousarray(wd[c]),
            }
        )
    return in_maps


def kernel(
    hidden_states,
    router_weight,
    w_gate,
    w_up,
    w_down,
    top_k,
    _trace: bool = False,
    _trace_all: bool = False,
    **_unused,
):
    assert int(top_k) == 2, "kernel hardcodes top_k=2"
    in_maps = _make_in_maps(hidden_states, router_weight, w_gate, w_up, w_down)
    nc = _build_nc()
    res = run_bass_kernel_spmd(
        nc, in_maps, core_ids=list(range(NCORES)), trace=_trace,
        trace_cores=list(range(NCORES)) if (_trace and _trace_all) else None,
    )
    outs = np.stack([res.results[c]["out"] for c in range(NCORES)], axis=0)
    out = outs.astype(np.float64).sum(axis=0).astype(np.float32)
    if _trace:
        kernel.last_exec_time_ns = res.exec_time_ns
        kernel.last_mean_exec_time_ns = res.mean_exec_time_ns
        kernel.last_trace = res.instructions_and_trace
    return out.reshape(B, S, H)


# revision 17
# speedup vs baseline: 1.0665x; 1.0665x over previous
"""MoE fused top-k-gating decode kernel for Trainium2 (8 NeuronCores).

Problem: B=32,S=1,H=2048, E=8 experts, I=5632, top_k=2, fp32.
Sharding: expert-parallel - core c owns expert c (w_gate/w_up/w_down[c]),
router weight replicated (rolled per-core so column 0 is the own
expert). Each core computes the full router (softmax + top-2 mask),
its expert's GLU-MLP for all 32 tokens scaled by its combine weight,
and returns a partial [T,H] fp16 output; the host sums the 8 partials.

DMA-bound at the fp16 weight roofline: 3*H*I*2B = 69.2 MB/core
@ ~360 GB/s = ~193 us floor (fp8 was evaluated and fails the 2e-2
accuracy gate; fp16 lands at ~6e-4). Key structure:
- gate/up slabs stream as ONE interleaved wgu DMA per 512-col slab
  (~2KB descriptors; small descriptors run at full rate, 16KB ones
  ~12% slower on real HW), double-buffered; matmuls keep the 32-token
  activations stationary and stream weights as the moving operand.
- swiglu epilogue: sigmoid reads PSUM directly; the top-2 combine
  weight folds into the intermediate, which lands fp16 and is
  32x32-block-transposed on the DVE into interT (PE queue stays pure
  matmul; freed PSUM banks double-buffer g/u so consecutive slabs
  overlap).
- down-weight chunks stream with a lag: groups 1..6 ride along slabs
  2..7 (filling the PE while each epilogue runs), groups 7..10 and
  finally chunks 0..3 (whose interT has been ready since slab 0)
  stream AFTER the last gate/up slab on the HWDGE sync/scalar queues,
  so the ~20us PE tail chain hides under ~29us of remaining weight
  traffic and only 4 matmuls + output copies trail the last byte.
- PSUM accumulation order per output bank: chunks 4..43 then 0..3
  (start at 4, stop at 3). Output copies alternate DVE/ACT (Copy
  table prewarmed during the router).
"""

import numpy as np

import concourse.bass as bass
import concourse.bacc as bacc
import concourse.mybir as mybir
import concourse.tile as tile
from concourse.masks import make_identity
from concourse.bass_utils import run_bass_kernel_spmd

B, S, H = 32, 1, 2048
E, I = 8, 5632
T = B * S
P = 128
NCORES = 8
SWIGLU_SCALE = 1.702

KH = H // P
KI = I // P
NW = 512
NT = I // NW
ND = H // NW
XW = T + E

F32 = mybir.dt.float32
F16 = mybir.dt.float16
AX = mybir.AxisListType.X
AF = mybir.ActivationFunctionType
OP = mybir.AluOpType


def _build_nc() -> bass.Bass:
    nc = bacc.Bacc()

    WDT = F16
    # xrw arrives host-packed in SBUF tile layout: one DMA with 128
    # contiguous 1.25KB descriptors (the [H, XW] layout's 80B descriptors
    # pay the <512B 2x latency penalty and hold the queue ~2us at boot)
    xrw_d = nc.declare_dram_parameter("xrw", [P, KH * XW], WDT, isOutput=False)
    # wg and wu interleaved per I-slab: one DMA + one completion semaphore
    # feeds both gate and up matmuls of a slab
    wgu_d = nc.declare_dram_parameter("wgu", [H, 2 * I], WDT, isOutput=False)
    wd_d = nc.declare_dram_parameter("wd", [I, H], WDT, isOutput=False)
    out_d = nc.declare_dram_parameter("out", [T, H], F16, isOutput=True)

    with tile.TileContext(nc) as tc:
        with tc.tile_pool(name="const", bufs=1) as const:
            xrw_sb = const.tile([P, KH * XW], WDT, name="xrw_sb")
            nc.sync.dma_start(out=xrw_sb, in_=xrw_d[:, :])

            interT_sb = const.tile([P, KI * T], WDT, name="interT_sb")
            comb_sb = const.tile([T, 1], F32, name="comb_sb")

            def xT_k(k):
                return xrw_sb[:, k * XW : k * XW + T]

            def rw_k(k):
                return xrw_sb[:, k * XW + T : (k + 1) * XW]

            wgu_cols = wgu_d.rearrange("(k p) i -> p k i", p=P)
            wd_rows = wd_d.rearrange("(q p) h -> p q h", p=P)
            wgp = tc.alloc_tile_pool(name="wgp", bufs=2)
            wdp = tc.alloc_tile_pool(name="wdp", bufs=3)

            with (
                tc.tile_pool(name="rps", bufs=1, space="PSUM") as rps,
                tc.tile_pool(name="rsb", bufs=1) as rsb,
            ):
                logits = rps.tile([T, E], F32, name="logits", tag="logits")
                for k in range(KH):
                    nc.tensor.matmul(
                        logits,
                        xT_k(k),
                        rw_k(k),
                        start=(k == 0),
                        stop=(k == KH - 1),
                    )
                lg = rsb.tile([T, E], F32, name="lg")
                nc.vector.tensor_copy(lg, logits)
                mx = rsb.tile([T, 1], F32, name="mx")
                nc.vector.reduce_max(mx, lg, axis=AX)
                nmx = rsb.tile([T, 1], F32, name="nmx")
                nc.vector.tensor_scalar_mul(nmx, mx, -1.0)
                ex = rsb.tile([T, E], F32, name="ex")
                nc.scalar.activation(ex, lg, AF.Exp, bias=nmx, scale=1.0)
                sm = rsb.tile([T, 1], F32, name="sm")
                nc.vector.reduce_sum(sm, ex, axis=AX)
                rc = rsb.tile([T, 1], F32, name="rc")
                nc.vector.reciprocal(rc, sm)
                aff = rsb.tile([T, E], F32, name="aff")
                nc.vector.tensor_scalar_mul(aff, ex, rc)
                m1 = rsb.tile([T, 1], F32, name="m1")
                nc.vector.reduce_max(m1, aff, axis=AX)
                pen = rsb.tile([T, E], F32, name="pen")
                nc.vector.tensor_scalar(
                    pen, aff, m1, -1e30, op0=OP.is_equal, op1=OP.mult
                )
                b2 = rsb.tile([T, E], F32, name="b2")
                nc.vector.tensor_add(b2, aff, pen)
                m2 = rsb.tile([T, 1], F32, name="m2")
                nc.vector.reduce_max(m2, b2, axis=AX)
                ge = rsb.tile([T, E], F32, name="ge")
                nc.vector.tensor_scalar(ge, aff, m2, None, op0=OP.is_ge)
                msk = rsb.tile([T, E], F32, name="msk")
                nc.vector.tensor_mul(msk, aff, ge)
                # ACT-engine Copy also prewarms its table for the tail copies
                nc.scalar.activation(comb_sb, msk[:, 0:1], AF.Copy, scale=1.0)

            with (
                tc.tile_pool(name="gup", bufs=2, space="PSUM") as gup,
                tc.tile_pool(name="dps", bufs=1, space="PSUM") as dps,
                tc.tile_pool(name="esb", bufs=2) as esb,
            ):
                d_ps = [
                    dps.tile([T, NW], F32, name=f"d_ps{j}", tag=f"d{j}")
                    for j in range(ND)
                ]

                def down_mms(ki, wd_ap, c):
                    # 4 matmuls accumulating chunk ki from wd_ap's chunk c.
                    # Accumulation order: 4..43, then 0..3 (start 4, stop 3).
                    for j in range(ND):
                        nc.tensor.matmul(
                            d_ps[j],
                            interT_sb[:, ki * T : (ki + 1) * T],
                            wd_ap[:, c * H + j * NW : c * H + (j + 1) * NW],
                            start=(ki == 4),
                            stop=(ki == 3),
                        )

                NW2 = 2 * NW
                for n in range(NT):
                    wgu_sl = wgp.tile([P, KH * NW2], WDT, name="wgu_sl", tag="wgu")
                    nc.sync.dma_start(
                        out=wgu_sl.rearrange("p (k c) -> p k c", c=NW2),
                        in_=wgu_cols[:, :, n * NW2 : (n + 1) * NW2],
                    )
                    g_ps = gup.tile([T, NW], F32, name="g_ps", tag="g")
                    u_ps = gup.tile([T, NW], F32, name="u_ps", tag="u")
                    for k in range(KH):
                        nc.tensor.matmul(
                            g_ps,
                            xT_k(k),
                            wgu_sl[:, k * NW2 : k * NW2 + NW],
                            start=(k == 0),
                            stop=(k == KH - 1),
                        )
                    for k in range(KH):
                        nc.tensor.matmul(
                            u_ps,
                            xT_k(k),
                            wgu_sl[:, k * NW2 + NW : (k + 1) * NW2],
                            start=(k == 0),
                            stop=(k == KH - 1),
                        )
                    # down matmuls for group n-1 (chunks 4(n-1)..4n-1) fill
                    # the PE while the epilogue chain runs on DVE/ACT.
                    # Groups 7..10 and 0 are deferred past the last slab.
                    if 2 <= n <= 7:
                        g = n - 1
                        for q in (0, 1):
                            k0 = 4 * g + 2 * q
                            wd_pr = wdp.tile(
                                [P, 2 * H], WDT, name="wd_pr", tag="wdpr"
                            )
                            nc.gpsimd.dma_start(
                                out=wd_pr.rearrange("p (q h) -> p q h", h=H),
                                in_=wd_rows[:, k0 : k0 + 2, :],
                            )
                            down_mms(k0, wd_pr, 0)
                            down_mms(k0 + 1, wd_pr, 1)
                    # epilogue: sigmoid reads PSUM directly; fold combine
                    # weight; inter lands fp16 and is block-transposed on
                    # the DVE straight into interT (no PE, no PSUM).
                    sig = esb.tile([T, NW], F32, name="sig", tag="sig")
                    nc.scalar.activation(
                        sig, g_ps, AF.Sigmoid, scale=SWIGLU_SCALE
                    )
                    t1 = esb.tile([T, NW], F32, name="t1", tag="t1")
                    nc.vector.tensor_mul(t1, g_ps, sig)
                    t2 = esb.tile([T, NW], F32, name="t2", tag="t2")
                    nc.vector.tensor_mul(t2, t1, u_ps)
                    inter = esb.tile([T, NW], WDT, name="inter", tag="inter")
                    nc.vector.tensor_scalar_mul(inter, t2, comb_sb)
                    for j in range(NW // P):
                        ic = 4 * n + j
                        for a in range(P // T):
                            nc.vector.transpose(
                                interT_sb[
                                    a * T : (a + 1) * T,
                                    ic * T : (ic + 1) * T,
                                ],
                                inter[:, j * P + a * T : j * P + (a + 1) * T],
                            )

                # tail: groups 7..10 as 4-chunk loads ping-ponging the
                # sync/scalar HWDGE queues (gpsimd's 1us software DGE per
                # instruction would trickle the last bytes in late), then
                # group 0 (chunks 0..3) last as two pairs.
                for i, g in enumerate((7, 8, 9, 10)):
                    wd_gr = wdp.tile([P, 4 * H], WDT, name="wd_gr", tag="wdgr")
                    eng = nc.sync if i % 2 == 0 else nc.scalar
                    eng.dma_start(
                        out=wd_gr.rearrange("p (q h) -> p q h", h=H),
                        in_=wd_rows[:, 4 * g : 4 * g + 4, :],
                    )
                    for c in range(4):
                        down_mms(4 * g + c, wd_gr, c)
                wd_pr = wdp.tile([P, 2 * H], WDT, name="wd_pr", tag="wdpr")
                nc.sync.dma_start(
                    out=wd_pr.rearrange("p (q h) -> p q h", h=H),
                    in_=wd_rows[:, 0:2, :],
                )
                down_mms(0, wd_pr, 0)
                down_mms(1, wd_pr, 1)
                # last two chunks as singles so only 4 matmuls + the output
                # copies trail the final weight byte
                wd_s2 = wdp.tile([P, H], WDT, name="wd_s2", tag="wds2")
                nc.scalar.dma_start(out=wd_s2, in_=wd_d[2 * P : 3 * P, :])
                down_mms(2, wd_s2, 0)
                wd_s3 = wdp.tile([P, H], WDT, name="wd_s3", tag="wds3")
                nc.sync.dma_start(out=wd_s3, in_=wd_d[3 * P : 4 * P, :])
                down_mms(3, wd_s3, 0)

                # output: PSUM -> fp16 SBUF (alternate DVE/ACT) -> DRAM
                out_sb = const.tile([T, H], F16, name="out_sb")
                for j in range(ND):
                    if j % 2 == 0:
                        nc.vector.tensor_copy(
                            out_sb[:, j * NW : (j + 1) * NW], d_ps[j]
                        )
                    else:
                        nc.scalar.activation(
                            out_sb[:, j * NW : (j + 1) * NW], d_ps[j],
                            AF.Copy, scale=1.0,
                        )
                    nc.sync.dma_start(
                        out=out_d[:, j * NW : (j + 1) * NW],
                        in_=out_sb[:, j * NW : (j + 1) * NW],
                    )
            wdp.release()
            wgp.release()
    nc.finalize()
    return nc


def _make_in_maps(hidden_states, router_weight, w_gate, w_up, w_down):
    x = np.asarray(hidden_states, np.float32).reshape(T, H)
    rw = np.asarray(router_weight, np.float32)
    wg = np.asarray(w_gate, np.float16)
    wu = np.asarray(w_up, np.float16)
    wd = np.ascontiguousarray(np.asarray(w_down, np.float16))
    # interleave gate/up slabs: wgu[H, 2I], slab n = [wg slab n | wu slab n]
    NWc = I // NT
    wgu = np.stack(
        [wg.reshape(E, H, NT, NWc), wu.reshape(E, H, NT, NWc)], axis=3
    ).reshape(E, H, 2 * I)
    xT = np.ascontiguousarray(x.T.astype(np.float16))
    in_maps = []
    for c in range(NCORES):
        order = [(j + c) % E for j in range(E)]
        rwT = rw[order].T.astype(np.float16)
        xrw = np.concatenate([xT, rwT], axis=1)  # [H, XW]
        # pack to SBUF tile layout [P, KH*XW]
        xrw = np.ascontiguousarray(
            xrw.reshape(KH, P, XW).transpose(1, 0, 2).reshape(P, KH * XW)
        )
        in_maps.append({
            "xrw": xrw,
            "wgu": np.ascontiguousarray(wgu[c]),
            "wd": wd[c],
        })
    return in_maps


def kernel(
    hidden_states,
    router_weight,
    w_gate,
    w_up,
    w_down,
    top_k,
    _trace: bool = False,
    _trace_all: bool = False,
    **_unused,
):
    assert int(top_k) == 2, "kernel hardcodes top_k=2"
    in_maps = _make_in_maps(hidden_states, router_weight, w_gate, w_up, w_down)
    nc = _build_nc()
    res = run_bass_kernel_spmd(
        nc, in_maps, core_ids=list(range(NCORES)), trace=_trace,
        trace_cores=list(range(NCORES)) if (_trace and _trace_all) else None,
    )
    outs = np.stack([res.results[c]["out"] for c in range(NCORES)], axis=0)
    out = outs.sum(axis=0, dtype=np.float64).astype(np.float32)
    if _trace:
        kernel.last_exec_time_ns = res.exec_time_ns
        kernel.last_mean_exec_time_ns = res.mean_exec_time_ns
        kernel.last_trace = res.instructions_and_trace
    return out.reshape(B, S, H)
